# revision 26
# baseline (speedup 1.0000x reference)
"""Distributed GQA attention prefill kernel for 8 TRN2 NeuronCores.

Problem: llama-style attention, BSZ=2, SEQ=2048, DIM=4096, 32 Q heads,
8 KV heads, head_dim=128, causal prefill (start_pos=0, caches zero).

Sharding: data-parallel over batch (2) x tensor-parallel over heads (4).
Core c = (b, g) with b = c // 4, g = c % 4 handles batch b, Q heads
8g..8g+7, KV heads 2g..2g+1, and wo rows 1024g..1024(g+1). Each core
emits a partial [2048, 4096] output (bf16); the host sums the 4 TP
partials per batch in f32. No collectives.

On-chip layout trick: everything is computed in "transposed" layouts so
no activation transpose is ever needed:
  QT[d, t] = wq.T @ x.T       (lhsT = wq natural, rhs = xT from host)
  KT[d, t] = wk.T @ x.T
  V[t, d]  = x @ wv           (lhsT = xT chunk, rhs = wv natural)
  scoresT[kv, q] = K @ QT     (lhsT = KT tile, rhs = QT tile)
  attn[q, d+1]   = P @ [V|1]  (lhsT = expT tile, rhs = V with ones col
                               -> last column accumulates the softmax
                               denominator for free)
RoPE is applied in rotate-half form: the head_dim of wq/wk is permuted
on the host (even dims first, odd dims second) which leaves all dot
products unchanged; cos/sin arrive transposed [64, t] in bf16.

Schedule:
  Stage A streams x in 8 windows of 256 tokens.  Within a window the
  K/V projections run k-chunk-outer so the PE consumes DMA chunks as
  they land; the Q projection of window w runs during window w+1 so
  the 8MB wq load never stalls the pipe.  QT spills to DRAM.
  Stage B software-pipelines heads: the scores+exp of head h are
  interleaved with the P@V of head h-1 and with out-projection matmuls
  of the previous q-tile's tokens, so ScalarE exp latency never idles
  the PE.  Causal masking is done by zeroing the masked triangle of
  the exp output on GpSimd (no DVE op between matmul and exp).
"""

import sys

for p in ("/opt/pypackages", "/opt/trn_rl_repo"):
    if p not in sys.path:
        sys.path.insert(0, p)

import numpy as np
import ml_dtypes

BSZ, SEQ, DIM = 2, 2048, 4096
N_HEADS, N_KV, HD = 32, 8, 128
H_L, KV_L = 8, 2          # per-core local Q heads / KV heads
HL = H_L * HD             # 1024 local head dims
N_CORES = 8
WIN = 256                 # stage-A token window
NW = SEQ // WIN
NK = DIM // 128
NQT = SEQ // 512          # attention q-tiles

_cache = {}


def _build():
    import concourse.mybir as mybir
    import concourse.tile as tile
    from concourse import bacc
    from concourse.masks import make_identity
    from contextlib import ExitStack

    f32 = mybir.dt.float32
    bf16 = mybir.dt.bfloat16
    Exp = mybir.ActivationFunctionType.Exp

    nc = bacc.Bacc()
    xT = nc.declare_dram_parameter("xT", [DIM, SEQ], bf16, isOutput=False)
    wq = nc.declare_dram_parameter("wq", [DIM, HL], bf16, isOutput=False)
    # wk arrives kh-major ([partition, kv_head, ko*hd]) so each kv head's
    # weights are one fully-contiguous 1MB DMA
    wk = nc.declare_dram_parameter("wk", [128, KV_L, NK * HD], bf16,
                                   isOutput=False)
    wv = nc.declare_dram_parameter("wv", [DIM, KV_L * HD], bf16, isOutput=False)
    wo = nc.declare_dram_parameter("wo", [HL, DIM], bf16, isOutput=False)
    cosT = nc.declare_dram_parameter("cosT", [64, SEQ], bf16, isOutput=False)
    sinT = nc.declare_dram_parameter("sinT", [64, SEQ], bf16, isOutput=False)
    out = nc.declare_dram_parameter("out", [SEQ, DIM], bf16, isOutput=True)

    qt_dram = nc.dram_tensor("qt_spill", [H_L, HD, SEQ], bf16)

    def dma_split(dst, src, n):
        """Issue n parallel DMAs over the ko axis (dim 1 of dst)."""
        ko = dst.shape[1]
        step = ko // n
        for i in range(n):
            nc.sync.dma_start(
                out=dst[:, i * step:(i + 1) * step],
                in_=src[:, i * step:(i + 1) * step])

    def dma_split_rr(dst, src, n):
        """n parallel DMAs with interleaved ko ranges: DMA i carries
        chunks i, i+n, i+2n ... so low-k chunks land first on every
        queue and the consumer (which walks k ascending) never waits
        behind a whole contiguous range."""
        for i in range(n):
            nc.sync.dma_start(out=dst[:, i::n], in_=src[:, i::n])

    with tile.TileContext(nc) as tc, ExitStack() as res:
        resid = res.enter_context(tc.tile_pool(name="resid", bufs=1))
        qt_pool = res.enter_context(tc.tile_pool(name="qt", bufs=3))
        qts = {}

        # K cache (rotated) [d, kv_tok] and V cache [kv_tok, d | ones]
        kt_all = resid.tile([128, KV_L, SEQ], bf16, tag="kt")
        v_all = resid.tile([128, SEQ // 128, KV_L, 130], bf16, tag="v")
        ident = resid.tile([128, 128], bf16, tag="ident")

        # ---- stage A: Q/K/V projection + RoPE --------------------------
        with ExitStack() as sa:
            psA = sa.enter_context(
                tc.tile_pool(name="psA", bufs=8, space="PSUM"))
            wq_sb = sa.enter_context(tc.tile_pool(name="wq", bufs=1)).tile(
                [128, NK, HL], bf16, tag="wq")
            wk_sb = sa.enter_context(tc.tile_pool(name="wk", bufs=1)).tile(
                [128, KV_L, NK * HD], bf16, tag="wk")
            wv_sb = sa.enter_context(tc.tile_pool(name="wv", bufs=1)).tile(
                [128, NK, KV_L * HD], bf16, tag="wv")
            cs_pool = sa.enter_context(tc.tile_pool(name="cs", bufs=1))
            cos_sb = cs_pool.tile([64, SEQ], bf16, tag="cos")
            sin_sb = cs_pool.tile([64, SEQ], bf16, tag="sin")
            xt_pool = sa.enter_context(tc.tile_pool(name="xt", bufs=3))
            rope_pool = sa.enter_context(tc.tile_pool(name="rope", bufs=2))
            qsp_pool = sa.enter_context(tc.tile_pool(name="qsp", bufs=2))

            def rope(ps, dst, t0, tw):
                """dst[0:64]=e*c-o*s ; dst[64:128]=e*s+o*c.
                One ScalarE copy stages the PSUM to bf16 SBUF: the PSUM
                bank frees after ~0.5us (instead of being held across
                four DVE reads) and the DVE math runs all-bf16 at 2x."""
                ste = rope_pool.tile([64, WIN], bf16, tag="ste",
                                     name="ste")[:, :tw]
                sto = rope_pool.tile([64, WIN], bf16, tag="sto",
                                     name="sto")[:, :tw]
                nc.scalar.copy(ste, ps[0:64, :tw])
                nc.scalar.copy(sto, ps[64:128, :tw])
                c = cos_sb[:, t0:t0 + tw]
                s = sin_sb[:, t0:t0 + tw]
                t1 = rope_pool.tile([64, WIN], bf16, tag="r1", name="r1")[:, :tw]
                t2 = rope_pool.tile([64, WIN], bf16, tag="r2", name="r2")[:, :tw]
                t3 = rope_pool.tile([64, WIN], bf16, tag="r3", name="r3")[:, :tw]
                t4 = rope_pool.tile([64, WIN], bf16, tag="r4", name="r4")[:, :tw]
                nc.vector.tensor_mul(t1, ste, c)
                nc.vector.tensor_mul(t2, sto, s)
                nc.vector.tensor_mul(t3, ste, s)
                nc.vector.tensor_mul(t4, sto, c)
                nc.vector.tensor_sub(dst[0:64, :tw], t1, t2)
                nc.vector.tensor_add(dst[64:128, :tw], t3, t4)

            # -- DMA priority chain: window-0 x + wk first, one DMA per
            # k-chunk in ascending consumption order so chunks spread
            # round-robin across queues and land in the order the PE
            # consumes them; then wv, cos/sin, window-1 x, then the 8MB
            # wq (consumed one window later).
            xt_tiles = {}
            xt0 = xt_pool.tile([128, NK, WIN], bf16, tag="xt", name="xt0")
            xt_tiles[0] = xt0
            xt0_src = xT[:, 0:WIN].rearrange("(ko p) t -> p ko t", p=128)
            wv_src = wv.rearrange("(ko p) d -> p ko d", p=128)
            nc.sync.dma_start(out=xt0[:, 0:2], in_=xt0_src[:, 0:2])
            nc.sync.dma_start(out=wk_sb[:, 0, 0:2 * HD], in_=wk[:, 0, 0:2 * HD])
            dma_split(xt0[:, 2:], xt0_src[:, 2:], 6)
            for i in range(3):
                nc.sync.dma_start(
                    out=wk_sb[:, 0, 2 * HD + i * 10 * HD:
                              min(NK * HD, 2 * HD + (i + 1) * 10 * HD)],
                    in_=wk[:, 0, 2 * HD + i * 10 * HD:
                           min(NK * HD, 2 * HD + (i + 1) * 10 * HD)])
            for i in range(3):
                nc.sync.dma_start(
                    out=wk_sb[:, 1, i * 11 * HD:min(NK * HD, (i + 1) * 11 * HD)],
                    in_=wk[:, 1, i * 11 * HD:min(NK * HD, (i + 1) * 11 * HD)])
            dma_split(wv_sb, wv_src, 4)
            nc.sync.dma_start(out=cos_sb, in_=cosT[:, :])
            nc.sync.dma_start(out=sin_sb, in_=sinT[:, :])
            xt1 = xt_pool.tile([128, NK, WIN], bf16, tag="xt", name="xt1")
            xt_tiles[1] = xt1
            dma_split(xt1, xT[:, WIN:2 * WIN].rearrange(
                "(ko p) t -> p ko t", p=128), 4)
            # wq in 4 column blocks (2 heads each) so Q of window 0 can
            # start as soon as the first block lands.
            wq_src = wq.rearrange("(ko p) d -> p ko d", p=128)
            for b in range(4):
                nc.sync.dma_start(
                    out=wq_sb[:, :, b * 256:(b + 1) * 256],
                    in_=wq_src[:, :, b * 256:(b + 1) * 256])
            make_identity(nc, ident)
            nc.vector.memset(v_all[:, :, :, 128:129], 1.0)
            # warm the ScalarE Exp table now so the first attention exp
            # doesn't pay the ~1.3us ACT_TABLE_LOAD mid-kernel
            warm = rope_pool.tile([128, 1], f32, tag="warm", name="warm")
            nc.vector.memset(warm, 0.0)
            nc.scalar.activation(warm, warm, Exp)

            def k_proj(xt, t0, kh_serial=False):
                # k-outer: consume each x/wk chunk as it lands.  Window 0
                # runs kh-serial instead so only kv-head 0's wk half (1MB)
                # gates the very first matmuls.
                pss = [psA.tile([128, WIN], f32, tag="psA", name=f"psk{kh}")
                       for kh in range(KV_L)]
                if kh_serial:
                    for kh in range(KV_L):
                        for k in range(NK):
                            nc.tensor.matmul(
                                pss[kh], wk_sb[:, kh, k * HD:(k + 1) * HD],
                                xt[:, k], start=(k == 0), stop=(k == NK - 1))
                        rope(pss[kh], kt_all[:, kh, t0:t0 + WIN], t0, WIN)
                    return
                for k in range(NK):
                    for kh in range(KV_L):
                        nc.tensor.matmul(
                            pss[kh], wk_sb[:, kh, k * HD:(k + 1) * HD],
                            xt[:, k], start=(k == 0), stop=(k == NK - 1))
                for kh in range(KV_L):
                    rope(pss[kh], kt_all[:, kh, t0:t0 + WIN], t0, WIN)

            def v_proj(xt, t0):
                pss = [psA.tile([128, KV_L * HD], f32, tag="psA",
                                name=f"psv{tc_}")
                       for tc_ in range(WIN // 128)]
                for k in range(NK):
                    for tc_ in range(WIN // 128):
                        nc.tensor.matmul(
                            pss[tc_], xt[:, k, tc_ * 128:(tc_ + 1) * 128],
                            wv_sb[:, k], start=(k == 0), stop=(k == NK - 1))
                for tc_ in range(WIN // 128):
                    for kh in range(KV_L):
                        nc.scalar.copy(
                            v_all[:, t0 // 128 + tc_, kh, 0:128],
                            pss[tc_][:, kh * HD:(kh + 1) * HD])

            def q_unit(xt, t0, h):
                ps = psA.tile([128, WIN], f32, tag="psA", name="psq")
                for k in range(NK):
                    nc.tensor.matmul(
                        ps, wq_sb[:, k, h * HD:(h + 1) * HD],
                        xt[:, k], start=(k == 0), stop=(k == NK - 1))
                qs = qsp_pool.tile([128, WIN], bf16, tag="qs", name="qs")
                rope(ps, qs, t0, WIN)
                nc.sync.dma_start(out=qt_dram[h, :, t0:t0 + WIN], in_=qs)

            for w in range(NW):
                t0 = w * WIN
                if w >= 1 and w + 1 < NW:
                    # prefetch next window's x one full window ahead
                    xt = xt_pool.tile([128, NK, WIN], bf16, tag="xt",
                                      name="xt")
                    xt_tiles[w + 1] = xt
                    dma_split(xt, xT[:, t0 + WIN:t0 + 2 * WIN].rearrange(
                        "(ko p) t -> p ko t", p=128), 4)
                # prefetch q-tile loads once their spill windows are done
                if w == 3:
                    qts[0] = qt_pool.tile([128, H_L, 512], bf16, tag="qt",
                                          name="qt0")
                    dma_split(qts[0], qt_dram[:, :, 0:512]
                              .rearrange("h p q -> p h q"), 2)
                if w == 5:
                    qts[1] = qt_pool.tile([128, H_L, 512], bf16, tag="qt",
                                          name="qt1")
                    dma_split(qts[1], qt_dram[:, :, 512:1024]
                              .rearrange("h p q -> p h q"), 2)
                k_proj(xt_tiles[w], t0, kh_serial=(w == 0))
                v_proj(xt_tiles[w], t0)
                if w >= 1:
                    for h in range(H_L):
                        q_unit(xt_tiles[w - 1], t0 - WIN, h)
                    del xt_tiles[w - 1]
            for h in range(H_L):
                q_unit(xt_tiles[NW - 1], SEQ - WIN, h)

        # ---- stage B: attention with out-projection interleaved --------
        with ExitStack() as bc:
            # creation order oc -> ap -> sc: the scores pool (used first,
            # right at the stage boundary) should land on the PSUM banks
            # freed earliest by the last stage-A q_units.
            ps_oc = bc.enter_context(
                tc.tile_pool(name="ps_oc", bufs=3, space="PSUM"))
            ps_ap = bc.enter_context(
                tc.tile_pool(name="ps_ap", bufs=2, space="PSUM"))
            ps_sc = bc.enter_context(
                tc.tile_pool(name="ps_sc", bufs=3, space="PSUM"))
            exp_pool = bc.enter_context(tc.tile_pool(name="exp", bufs=32))
            asb_pool = bc.enter_context(tc.tile_pool(name="asb", bufs=8))
            rec_pool = bc.enter_context(tc.tile_pool(name="rec", bufs=8))
            at_sb = bc.enter_context(tc.tile_pool(name="at", bufs=1)).tile(
                [128, H_L, SEQ], bf16, tag="at")
            wo_pool = bc.enter_context(tc.tile_pool(name="wo", bufs=2))
            out_pool = bc.enter_context(tc.tile_pool(name="outp", bufs=4))

            wo_cur = [None]
            cqueue = []

            def make_strip(qs_):
                """Emission closures for out-proj of token strip qs_.
                wo tile loads run one di-block ahead of their consumers."""
                def load_wo(di):
                    wot = wo_pool.tile([128, H_L, 512], bf16, tag="wo",
                                       name="wot")
                    dma_split(wot, wo[:, di * 512:(di + 1) * 512].rearrange(
                        "(ho p) d -> p ho d", p=128), 2)
                    return wot

                wo_ring = [None, None]

                cls = []

                def first_load():
                    wo_ring[0] = load_wo(0)
                cls.append(first_load)
                for di in range(DIM // 512):
                    if di + 1 < DIM // 512:
                        def next_load(di=di):
                            wo_ring[(di + 1) % 2] = load_wo(di + 1)
                        cls.append(next_load)
                    for tj in range(4):
                        def pair(di=di, ti=qs_ * 4 + tj):
                            wot = wo_ring[di % 2]
                            ps = ps_oc.tile([128, 512], f32, tag="oc",
                                            name="pso")
                            for ho in range(H_L):
                                nc.tensor.matmul(
                                    ps, at_sb[:, ho, ti * 128:(ti + 1) * 128],
                                    wot[:, ho], start=(ho == 0),
                                    stop=(ho == H_L - 1))
                            osb = out_pool.tile([128, 512], bf16, tag="osb",
                                                name="osb")
                            nc.scalar.copy(osb, ps)
                            nc.sync.dma_start(
                                out=out[ti * 128:(ti + 1) * 128,
                                        di * 512:(di + 1) * 512],
                                in_=osb)
                        cls.append(pair)
                return cls

            def pop_fill(n):
                for _ in range(n):
                    if cqueue:
                        cqueue.pop(0)()

            def make_scores(qi, h):
                """Emit per-kvt closures: scores MM + exp + causal zero."""
                q0 = qi * 512
                kh = h // 4
                qt = qts[qi]
                pes = []
                cls = []
                for kvt in range(4 * (qi + 1)):
                    r = kvt - 4 * qi
                    c0 = max(r, 0) * 128
                    pe = exp_pool.tile([128, 512], bf16, tag="exp", name="pe")
                    pes.append(pe)

                    def sc(kvt=kvt, r=r, c0=c0, pe=pe):
                        ps = ps_sc.tile([128, 512], f32, tag="sc", name="pss")
                        nc.tensor.matmul(
                            ps[:, c0:],
                            kt_all[:, kh, kvt * 128:(kvt + 1) * 128],
                            qt[:, h, c0:], start=True, stop=True)
                        nc.scalar.activation(pe[:, c0:], ps[:, c0:], Exp)
                        if r >= 0:
                            # zero the causally-masked upper triangle of
                            # the diagonal 128x128 block (q < kv)
                            nc.gpsimd.affine_select(
                                out=pe[:, c0:c0 + 128],
                                in_=pe[:, c0:c0 + 128],
                                pattern=[[1, 128]],
                                compare_op=mybir.AluOpType.is_ge,
                                fill=0.0, base=0, channel_multiplier=-1)
                    cls.append(sc)
                return pes, cls

            def make_pv(qi, h, pes):
                """Per-qc closures: P@V chain + normalize; then transposes."""
                q0 = qi * 512
                kh = h // 4
                asbs = []
                cls = []
                for qc in range(4):
                    def pv(qc=qc):
                        ap = ps_ap.tile([128, 129], f32, tag="ap", name="ap")
                        last = 4 * qi + qc
                        for kvt in range(last + 1):
                            nc.tensor.matmul(
                                ap, pes[kvt][:, qc * 128:(qc + 1) * 128],
                                v_all[:, kvt, kh, 0:129],
                                start=(kvt == 0), stop=(kvt == last))
                        rec = rec_pool.tile([128, 1], f32, tag="rec",
                                            name="rec")
                        nc.vector.reciprocal(rec, ap[:, 128:129])
                        asb = asb_pool.tile([128, 128], bf16, tag="asb",
                                            name="asb")
                        nc.vector.tensor_scalar_mul(asb, ap[:, 0:128], rec)
                        asbs.append(asb)
                    cls.append(pv)

                def transp():
                    for qc in range(4):
                        pst = ps_oc.tile([128, 128], bf16, tag="oc",
                                         name="pst")
                        nc.tensor.transpose(pst, asbs[qc], ident)
                        nc.vector.tensor_copy(
                            at_sb[:, h, q0 + qc * 128:q0 + (qc + 1) * 128],
                            pst)
                return cls, transp

            pend_pv = None   # (pv closures, transp closure) of prev head

            for qi in range(NQT):
                if qi + 2 < NQT:
                    nq = qi + 2
                    qts[nq] = qt_pool.tile([128, H_L, 512], bf16,
                                           tag="qt", name="qtn")
                    dma_split(qts[nq], qt_dram[:, :, nq * 512:nq * 512 + 512]
                              .rearrange("h p q -> p h q"), 2)
                for h in range(H_L):
                    if h == 0 and qi > 0:
                        # make the strip available now but only pop its two
                        # wo prefetch loads (pure DMA); its out-proj matmuls
                        # must wait for head 7's transposes below.
                        cqueue.extend(make_strip(qi - 1))
                        pop_fill(2)
                    fill_ok = h > 0 or qi == 0
                    pes, scs = make_scores(qi, h)
                    pvs = list(pend_pv[0]) if pend_pv else []
                    # interleave: scores tiles paced by exp; PV chains and
                    # out-proj fillers of the previous strip fill the PE
                    # while ScalarE catches up.
                    n = len(scs)
                    s = max(1, (n - 2) // 4)
                    pv_at = {2, 2 + s, 2 + 2 * s, 2 + 3 * s}
                    for i, sc in enumerate(scs):
                        sc()
                        if i in pv_at and pvs:
                            pvs.pop(0)()
                            if fill_ok:
                                pop_fill(1)
                    for pv in pvs:
                        pv()
                        if fill_ok:
                            pop_fill(1)
                    if pend_pv:
                        pend_pv[1]()   # transposes of previous head
                    pop_fill(2 if fill_ok else 0)
                    pend_pv = make_pv(qi, h, pes)
            # drain: last head's PV + transposes, then final strip
            for pv in pend_pv[0]:
                pv()
                pop_fill(2)
            pend_pv[1]()
            cqueue.extend(make_strip(NQT - 1))
            while cqueue:
                cqueue.pop(0)()

    nc.finalize()
    return nc


def _prep_inputs(x, wq, wk, wv, wo, freqs_cos, freqs_sin):
    """Host-side shard prep. Returns in_maps for cores 0..7."""
    bf = ml_dtypes.bfloat16
    perm = np.concatenate([np.arange(0, HD, 2), np.arange(1, HD, 2)])  # rotate-half

    wq_p = (wq.astype(np.float32) / np.sqrt(HD)).reshape(DIM, N_HEADS, HD)[:, :, perm]
    wk_p = wk.astype(np.float32).reshape(DIM, N_KV, HD)[:, :, perm]

    cosT = np.ascontiguousarray(freqs_cos.astype(np.float32).T).astype(bf)
    sinT = np.ascontiguousarray(freqs_sin.astype(np.float32).T).astype(bf)

    xTs = [np.ascontiguousarray(x[b].astype(np.float32).T).astype(bf)
           for b in range(BSZ)]

    in_maps = []
    for c in range(N_CORES):
        b, g = c // 4, c % 4
        in_maps.append({
            "xT": xTs[b],
            "wq": np.ascontiguousarray(
                wq_p[:, g * H_L:(g + 1) * H_L].reshape(DIM, HL)).astype(bf),
            "wk": np.ascontiguousarray(
                wk_p[:, g * KV_L:(g + 1) * KV_L]
                .reshape(NK, 128, KV_L, HD).transpose(1, 2, 0, 3)
                .reshape(128, KV_L, NK * HD)).astype(bf),
            "wv": np.ascontiguousarray(
                wv[:, g * KV_L * HD:(g + 1) * KV_L * HD]).astype(bf),
            "wo": np.ascontiguousarray(
                wo[g * HL:(g + 1) * HL]).astype(bf),
            "cosT": cosT,
            "sinT": sinT,
        })
    return in_maps


def _run(inputs, trace=False):
    from concourse.bass_utils import run_bass_kernel_spmd

    if "nc" not in _cache:
        _cache["nc"] = _build()
    nc = _cache["nc"]

    in_maps = _prep_inputs(
        np.asarray(inputs["x"]), np.asarray(inputs["wq"]),
        np.asarray(inputs["wk"]), np.asarray(inputs["wv"]),
        np.asarray(inputs["wo"]), np.asarray(inputs["freqs_cos"]),
        np.asarray(inputs["freqs_sin"]))

    res = run_bass_kernel_spmd(nc, in_maps, core_ids=list(range(N_CORES)),
                               trace=trace)
    out = np.zeros((BSZ, SEQ, DIM), np.float32)
    for c in range(N_CORES):
        out[c // 4] += res.results[c]["out"].astype(np.float32)
    return out, res


def kernel(**inputs) -> np.ndarray:
    out, _ = _run(inputs, trace=False)
    return out


# revision 44
# speedup vs baseline: 1.1782x; 1.1782x over previous
"""Distributed GQA attention prefill kernel for 8 TRN2 NeuronCores.

Problem: llama-style attention, BSZ=2, SEQ=2048, DIM=4096, 32 Q heads,
8 KV heads, head_dim=128, causal prefill (start_pos=0, caches zero).

Sharding: data-parallel over batch (2) x tensor-parallel over heads (4).
Core c = (b, g) with b = c // 4, g = c % 4 handles batch b, Q heads
8g..8g+7, KV heads 2g..2g+1, and wo rows 1024g..1024(g+1). Each core
emits a partial [2048, 4096] output (bf16); the host sums the 4 TP
partials per batch in f32. No collectives.

On-chip layout trick: everything is computed in "transposed" layouts so
no activation transpose is ever needed:
  QT[d, t] = wq.T @ x.T       (lhsT = wq natural, rhs = xT from host)
  KT[d, t] = wk.T @ x.T
  V[t, d]  = x @ wv           (lhsT = xT chunk, rhs = wv natural)
  scoresT[kv, q] = K @ QT     (lhsT = KT tile, rhs = QT tile)
  attn[q, d+1]   = P @ [V|1]  (lhsT = expT tile, rhs = V with ones col
                               -> last column accumulates the softmax
                               denominator for free)
RoPE is applied in rotate-half form: the head_dim of wq/wk is permuted
on the host (even dims first, odd dims second) which leaves all dot
products unchanged; cos/sin arrive transposed [64, t] in bf16.

Schedule:
  Stage A streams x in 8 windows of 256 tokens.  Within a window the
  K/V projections run k-chunk-outer so the PE consumes DMA chunks as
  they land; the Q projection of window w runs during window w+1 so
  the 8MB wq load never stalls the pipe.  QT spills to DRAM.
  Stage B software-pipelines heads: the scores+exp of head h are
  interleaved with the P@V of head h-1 and with out-projection matmuls
  of the previous q-tile's tokens, so ScalarE exp latency never idles
  the PE.  Causal masking is done by zeroing the masked triangle of
  the exp output on GpSimd (no DVE op between matmul and exp).
"""

import sys

for p in ("/opt/pypackages", "/opt/trn_rl_repo"):
    if p not in sys.path:
        sys.path.insert(0, p)

import numpy as np
import ml_dtypes

BSZ, SEQ, DIM = 2, 2048, 4096
N_HEADS, N_KV, HD = 32, 8, 128
H_L, KV_L = 8, 2          # per-core local Q heads / KV heads
HL = H_L * HD             # 1024 local head dims
N_CORES = 8
WIN = 256                 # stage-A K/V sub-window (pair 0 startup)
PW = 512                  # stage-A window pair (Q/K matmul free dim)
NP = SEQ // PW
NK = DIM // 128
NQT = SEQ // 512          # attention q-tiles

_cache = {}


def _build():
    import concourse.mybir as mybir
    import concourse.tile as tile
    from concourse import bacc
    from concourse.masks import make_identity
    from contextlib import ExitStack

    f32 = mybir.dt.float32
    bf16 = mybir.dt.bfloat16
    Exp = mybir.ActivationFunctionType.Exp

    nc = bacc.Bacc()
    xT = nc.declare_dram_parameter("xT", [DIM, SEQ], bf16, isOutput=False)
    wq = nc.declare_dram_parameter("wq", [DIM, HL], bf16, isOutput=False)
    # wk arrives kh-major ([partition, kv_head, ko*hd]) so each kv head's
    # weights are one fully-contiguous 1MB DMA
    wk = nc.declare_dram_parameter("wk", [128, KV_L, NK * HD], bf16,
                                   isOutput=False)
    wv = nc.declare_dram_parameter("wv", [DIM, KV_L * HD], bf16, isOutput=False)
    wo = nc.declare_dram_parameter("wo", [HL, DIM], bf16, isOutput=False)
    cosT = nc.declare_dram_parameter("cosT", [64, SEQ], bf16, isOutput=False)
    sinT = nc.declare_dram_parameter("sinT", [64, SEQ], bf16, isOutput=False)
    out = nc.declare_dram_parameter("out", [SEQ, DIM], bf16, isOutput=True)

    qt_dram = nc.dram_tensor("qt_spill", [H_L, HD, SEQ], bf16)

    def dma_split(dst, src, n):
        """Issue n parallel DMAs over the ko axis (dim 1 of dst)."""
        ko = dst.shape[1]
        assert ko % n == 0, f"dma_split: {ko} not divisible by {n}"
        step = ko // n
        for i in range(n):
            nc.sync.dma_start(
                out=dst[:, i * step:(i + 1) * step],
                in_=src[:, i * step:(i + 1) * step])

    def dma_split_rr(dst, src, n):
        """n parallel DMAs with interleaved ko ranges: DMA i carries
        chunks i, i+n, i+2n ... so low-k chunks land first on every
        queue and the consumer (which walks k ascending) never waits
        behind a whole contiguous range."""
        for i in range(n):
            nc.sync.dma_start(out=dst[:, i::n], in_=src[:, i::n])

    with tile.TileContext(nc) as tc, ExitStack() as res:
        resid = res.enter_context(tc.tile_pool(name="resid", bufs=1))
        qt_pool = res.enter_context(tc.tile_pool(name="qt", bufs=2))
        qts = {}

        # K cache (rotated) [d, kv_tok] and V cache [kv_tok, d | ones]
        kt_all = resid.tile([128, KV_L, SEQ], bf16, tag="kt")
        v_all = resid.tile([128, SEQ // 128, KV_L, 130], bf16, tag="v")
        ident = resid.tile([128, 128], bf16, tag="ident")

        # ---- stage A: Q/K/V projection + RoPE --------------------------
        with ExitStack() as sa:
            psA = sa.enter_context(
                tc.tile_pool(name="psA", bufs=8, space="PSUM"))
            wq_sb = sa.enter_context(tc.tile_pool(name="wq", bufs=1)).tile(
                [128, NK, HL], bf16, tag="wq")
            wk_sb = sa.enter_context(tc.tile_pool(name="wk", bufs=1)).tile(
                [128, KV_L, NK * HD], bf16, tag="wk")
            wv_sb = sa.enter_context(tc.tile_pool(name="wv", bufs=1)).tile(
                [128, NK, KV_L * HD], bf16, tag="wv")
            cs_pool = sa.enter_context(tc.tile_pool(name="cs", bufs=1))
            cos_sb = cs_pool.tile([64, SEQ], bf16, tag="cos")
            sin_sb = cs_pool.tile([64, SEQ], bf16, tag="sin")
            xt_pool = sa.enter_context(tc.tile_pool(name="xt", bufs=2))
            rope_pool = sa.enter_context(tc.tile_pool(name="rope", bufs=1))
            qsp_pool = sa.enter_context(tc.tile_pool(name="qsp", bufs=2))

            def rope(ps, dst, t0, tw):
                """dst[0:64]=e*c-o*s ; dst[64:128]=e*s+o*c.
                One ScalarE copy stages the PSUM to bf16 SBUF: the PSUM
                bank frees after ~0.5us (instead of being held across
                four DVE reads) and the DVE math runs all-bf16 at 2x."""
                ste = rope_pool.tile([64, PW], bf16, tag="ste",
                                     name="ste")[:, :tw]
                sto = rope_pool.tile([64, PW], bf16, tag="sto",
                                     name="sto")[:, :tw]
                nc.scalar.copy(ste, ps[0:64, :tw])
                nc.scalar.copy(sto, ps[64:128, :tw])
                c = cos_sb[:, t0:t0 + tw]
                s = sin_sb[:, t0:t0 + tw]
                t1 = rope_pool.tile([64, PW], bf16, tag="r1", name="r1")[:, :tw]
                t2 = rope_pool.tile([64, PW], bf16, tag="r2", name="r2")[:, :tw]
                nc.vector.tensor_mul(t1, ste, c)
                nc.vector.tensor_mul(t2, sto, s)
                nc.vector.tensor_sub(dst[0:64, :tw], t1, t2)
                # r1/r2 slots reused: allocate only after the sub above is
                # emitted so the WAR dependency protects t1/t2
                t3 = rope_pool.tile([64, PW], bf16, tag="r1", name="r3")[:, :tw]
                t4 = rope_pool.tile([64, PW], bf16, tag="r2", name="r4")[:, :tw]
                nc.vector.tensor_mul(t3, ste, s)
                nc.vector.tensor_mul(t4, sto, c)
                nc.vector.tensor_add(dst[64:128, :tw], t3, t4)

            # -- DMA priority chain: pair-0's first token half + wk
            # kv-head 0 lead so the first matmuls start ASAP, then the
            # rest of pair 0, wv, cos/sin, pair 1, then the 8MB wq.
            xt_tiles = {}
            xt0 = xt_pool.tile([128, NK, PW], bf16, tag="xt", name="xt0")
            xt_tiles[0] = xt0
            xt0_src = xT[:, 0:PW].rearrange("(ko p) t -> p ko t", p=128)
            wv_src = wv.rearrange("(ko p) d -> p ko d", p=128)
            nc.sync.dma_start(out=xt0[:, 0:2, 0:WIN],
                              in_=xt0_src[:, 0:2, 0:WIN])
            nc.sync.dma_start(out=wk_sb[:, 0, 0:2 * HD], in_=wk[:, 0, 0:2 * HD])
            dma_split(xt0[:, 2:, 0:WIN], xt0_src[:, 2:, 0:WIN], 6)
            for i in range(3):
                nc.sync.dma_start(
                    out=wk_sb[:, 0, 2 * HD + i * 10 * HD:
                              min(NK * HD, 2 * HD + (i + 1) * 10 * HD)],
                    in_=wk[:, 0, 2 * HD + i * 10 * HD:
                           min(NK * HD, 2 * HD + (i + 1) * 10 * HD)])
            dma_split(xt0[:, :, WIN:PW], xt0_src[:, :, WIN:PW], 4)
            for i in range(3):
                nc.sync.dma_start(
                    out=wk_sb[:, 1, i * 11 * HD:min(NK * HD, (i + 1) * 11 * HD)],
                    in_=wk[:, 1, i * 11 * HD:min(NK * HD, (i + 1) * 11 * HD)])
            dma_split(wv_sb, wv_src, 4)
            nc.sync.dma_start(out=cos_sb, in_=cosT[:, :])
            nc.sync.dma_start(out=sin_sb, in_=sinT[:, :])
            xt1 = xt_pool.tile([128, NK, PW], bf16, tag="xt", name="xt1")
            xt_tiles[1] = xt1
            dma_split(xt1, xT[:, PW:2 * PW].rearrange(
                "(ko p) t -> p ko t", p=128), 8)
            # wq in 4 column blocks (2 heads each) so Q of pair 0 can
            # start as soon as the first block lands.
            wq_src = wq.rearrange("(ko p) d -> p ko d", p=128)
            for b in range(4):
                nc.sync.dma_start(
                    out=wq_sb[:, :, b * 256:(b + 1) * 256],
                    in_=wq_src[:, :, b * 256:(b + 1) * 256])
            make_identity(nc, ident)
            nc.vector.memset(v_all[:, :, :, 128:129], 1.0)
            # warm the ScalarE Exp table now so the first attention exp
            # doesn't pay the ~1.3us ACT_TABLE_LOAD mid-kernel
            warm = rope_pool.tile([128, 1], f32, tag="warm", name="warm")
            nc.vector.memset(warm, 0.0)
            nc.scalar.activation(warm, warm, Exp)

            def k_proj_halves(xt, t0):
                """Pair 0 K: kh-serial, token-half-serial (N=256 chains,
                separate banks per half) so only x-half-0 + wk-head-0
                gate the very first matmuls."""
                for kh in range(KV_L):
                    for half in range(2):
                        c0 = half * WIN
                        ps = psA.tile([128, WIN], f32, tag="psA",
                                      name=f"psk{kh}{half}")
                        for k in range(NK):
                            nc.tensor.matmul(
                                ps, wk_sb[:, kh, k * HD:(k + 1) * HD],
                                xt[:, k, c0:c0 + WIN],
                                start=(k == 0), stop=(k == NK - 1))
                        rope(ps, kt_all[:, kh, t0 + c0:t0 + c0 + WIN],
                             t0 + c0, WIN)

            def k_proj(xt, t0):
                # k-outer at N=512: consume each x/wk chunk as it lands
                pss = [psA.tile([128, PW], f32, tag="psA", name=f"psk{kh}")
                       for kh in range(KV_L)]
                for k in range(NK):
                    for kh in range(KV_L):
                        nc.tensor.matmul(
                            pss[kh], wk_sb[:, kh, k * HD:(k + 1) * HD],
                            xt[:, k], start=(k == 0), stop=(k == NK - 1))
                for kh in range(KV_L):
                    rope(pss[kh], kt_all[:, kh, t0:t0 + PW], t0, PW)

            def v_proj(xt, t0):
                pss = [psA.tile([128, KV_L * HD], f32, tag="psA",
                                name=f"psv{tc_}")
                       for tc_ in range(PW // 128)]
                for k in range(NK):
                    for tc_ in range(PW // 128):
                        nc.tensor.matmul(
                            pss[tc_], xt[:, k, tc_ * 128:(tc_ + 1) * 128],
                            wv_sb[:, k], start=(k == 0), stop=(k == NK - 1))
                for tc_ in range(PW // 128):
                    for kh in range(KV_L):
                        nc.scalar.copy(
                            v_all[:, t0 // 128 + tc_, kh, 0:128],
                            pss[tc_][:, kh * HD:(kh + 1) * HD])

            def q_unit(xt, t0, h):
                ps = psA.tile([128, PW], f32, tag="psA", name="psq")
                for k in range(NK):
                    nc.tensor.matmul(
                        ps, wq_sb[:, k, h * HD:(h + 1) * HD],
                        xt[:, k], start=(k == 0), stop=(k == NK - 1))
                qs = qsp_pool.tile([128, PW], bf16, tag="qs", name="qs")
                rope(ps, qs, t0, PW)
                nc.sync.dma_start(out=qt_dram[h, :, t0:t0 + PW], in_=qs)

            for p in range(NP):
                t0 = p * PW
                if p >= 1 and p + 1 < NP:
                    # prefetch next pair's x one full pair ahead
                    xt = xt_pool.tile([128, NK, PW], bf16, tag="xt",
                                      name="xt")
                    xt_tiles[p + 1] = xt
                    dma_split(xt, xT[:, t0 + PW:t0 + 2 * PW].rearrange(
                        "(ko p) t -> p ko t", p=128), 8)
                if p == 1:
                    # pair 0's Q spill is complete -> prefetch its q-tile
                    qts[0] = qt_pool.tile([128, H_L, 512], bf16,
                                          tag="qt", name="qt0")
                    dma_split(qts[0], qt_dram[:, :, 0:PW]
                              .rearrange("h p q -> p h q"), 2)
                if p == 0:
                    k_proj_halves(xt_tiles[p], t0)
                else:
                    k_proj(xt_tiles[p], t0)
                v_proj(xt_tiles[p], t0)
                for h in range(H_L):
                    q_unit(xt_tiles[p], t0, h)
                if p >= 1:
                    del xt_tiles[p - 1]

        # ---- stage B: attention with out-projection interleaved --------
        with ExitStack() as bc:
            # creation order oc -> ap -> sc: the scores pool (used first,
            # right at the stage boundary) should land on the PSUM banks
            # freed earliest by the last stage-A q_units.
            ps_oc = bc.enter_context(
                tc.tile_pool(name="ps_oc", bufs=3, space="PSUM"))
            ps_ap = bc.enter_context(
                tc.tile_pool(name="ps_ap", bufs=2, space="PSUM"))
            ps_sc = bc.enter_context(
                tc.tile_pool(name="ps_sc", bufs=3, space="PSUM"))
            exp_pool = bc.enter_context(tc.tile_pool(name="exp", bufs=32))
            asb_pool = bc.enter_context(tc.tile_pool(name="asb", bufs=8))
            rec_pool = bc.enter_context(tc.tile_pool(name="rec", bufs=8))
            at_sb = bc.enter_context(tc.tile_pool(name="at", bufs=1)).tile(
                [128, H_L, SEQ], bf16, tag="at")
            wo_pool = bc.enter_context(tc.tile_pool(name="wo", bufs=2))
            out_pool = bc.enter_context(tc.tile_pool(name="outp", bufs=4))

            wo_cur = [None]
            cqueue = []

            def make_strip(qs_):
                """Emission closures for out-proj of token strip qs_.
                wo tile loads run one di-block ahead of their consumers."""
                def load_wo(di):
                    wot = wo_pool.tile([128, H_L, 512], bf16, tag="wo",
                                       name="wot")
                    dma_split(wot, wo[:, di * 512:(di + 1) * 512].rearrange(
                        "(ho p) d -> p ho d", p=128), 2)
                    return wot

                wo_ring = [None, None]

                cls = []

                def first_load():
                    wo_ring[0] = load_wo(0)
                cls.append(first_load)
                for di in range(DIM // 512):
                    if di + 1 < DIM // 512:
                        def next_load(di=di):
                            wo_ring[(di + 1) % 2] = load_wo(di + 1)
                        cls.append(next_load)
                    for tj in range(4):
                        def pair(di=di, ti=qs_ * 4 + tj):
                            wot = wo_ring[di % 2]
                            ps = ps_oc.tile([128, 512], f32, tag="oc",
                                            name="pso")
                            for ho in range(H_L):
                                nc.tensor.matmul(
                                    ps, at_sb[:, ho, ti * 128:(ti + 1) * 128],
                                    wot[:, ho], start=(ho == 0),
                                    stop=(ho == H_L - 1))
                            osb = out_pool.tile([128, 512], bf16, tag="osb",
                                                name="osb")
                            nc.scalar.copy(osb, ps)
                            nc.sync.dma_start(
                                out=out[ti * 128:(ti + 1) * 128,
                                        di * 512:(di + 1) * 512],
                                in_=osb)
                        cls.append(pair)
                return cls

            def pop_fill(n):
                for _ in range(n):
                    if cqueue:
                        cqueue.pop(0)()

            def make_scores(qi, h):
                """Emit per-kvt closures: scores MM + exp + causal zero."""
                q0 = qi * 512
                kh = h // 4
                qt = qts[qi]
                pes = []
                cls = []
                for kvt in range(4 * (qi + 1)):
                    r = kvt - 4 * qi
                    c0 = max(r, 0) * 128
                    pe = exp_pool.tile([128, 512], bf16, tag="exp", name="pe")
                    pes.append(pe)

                    def sc(kvt=kvt, r=r, c0=c0, pe=pe):
                        ps = ps_sc.tile([128, 512], f32, tag="sc", name="pss")
                        nc.tensor.matmul(
                            ps[:, c0:],
                            kt_all[:, kh, kvt * 128:(kvt + 1) * 128],
                            qt[:, h, c0:], start=True, stop=True)
                        nc.scalar.activation(pe[:, c0:], ps[:, c0:], Exp)
                        if r >= 0:
                            # zero the causally-masked upper triangle of
                            # the diagonal 128x128 block (q < kv)
                            nc.gpsimd.affine_select(
                                out=pe[:, c0:c0 + 128],
                                in_=pe[:, c0:c0 + 128],
                                pattern=[[1, 128]],
                                compare_op=mybir.AluOpType.is_ge,
                                fill=0.0, base=0, channel_multiplier=-1)
                    cls.append(sc)
                return pes, cls

            def make_pv(qi, h, pes):
                """Per-qc closures: P@V chain + normalize; then transposes."""
                q0 = qi * 512
                kh = h // 4
                asbs = []
                cls = []
                for qc in range(4):
                    def pv(qc=qc):
                        ap = ps_ap.tile([128, 129], f32, tag="ap", name="ap")
                        last = 4 * qi + qc
                        for kvt in range(last + 1):
                            nc.tensor.matmul(
                                ap, pes[kvt][:, qc * 128:(qc + 1) * 128],
                                v_all[:, kvt, kh, 0:129],
                                start=(kvt == 0), stop=(kvt == last))
                        rec = rec_pool.tile([128, 1], f32, tag="rec",
                                            name="rec")
                        nc.vector.reciprocal(rec, ap[:, 128:129])
                        asb = asb_pool.tile([128, 128], bf16, tag="asb",
                                            name="asb")
                        nc.vector.tensor_scalar_mul(asb, ap[:, 0:128], rec)
                        asbs.append(asb)
                    cls.append(pv)

                def transp():
                    for qc in range(4):
                        pst = ps_oc.tile([128, 128], bf16, tag="oc",
                                         name="pst")
                        nc.tensor.transpose(pst, asbs[qc], ident)
                        nc.vector.tensor_copy(
                            at_sb[:, h, q0 + qc * 128:q0 + (qc + 1) * 128],
                            pst)
                return cls, transp

            pend_pv = None   # (pv closures, transp closure) of prev head

            for qi in range(NQT):
                if qi + 1 < NQT:
                    # prefetch the next q-tile during this tile's attention
                    qts[qi + 1] = qt_pool.tile([128, H_L, 512], bf16,
                                               tag="qt", name="qtn")
                    dma_split(qts[qi + 1],
                              qt_dram[:, :, (qi + 1) * PW:(qi + 2) * PW]
                              .rearrange("h p q -> p h q"), 2)
                for h in range(H_L):
                    if h == 0 and qi > 0:
                        # make the strip available now but only pop its two
                        # wo prefetch loads (pure DMA); its out-proj matmuls
                        # must wait for head 7's transposes below.
                        cqueue.extend(make_strip(qi - 1))
                        pop_fill(2)
                    fill_ok = h > 0 or qi == 0
                    pes, scs = make_scores(qi, h)
                    pvs = list(pend_pv[0]) if pend_pv else []
                    # interleave: scores tiles paced by exp; PV chains and
                    # out-proj fillers of the previous strip fill the PE
                    # while ScalarE catches up.
                    n = len(scs)
                    s = max(1, (n - 2) // 4)
                    pv_at = {2, 2 + s, 2 + 2 * s, 2 + 3 * s}
                    for i, sc in enumerate(scs):
                        sc()
                        if i in pv_at and pvs:
                            pvs.pop(0)()
                            if fill_ok:
                                pop_fill(1)
                    for pv in pvs:
                        pv()
                        if fill_ok:
                            pop_fill(1)
                    if pend_pv:
                        pend_pv[1]()   # transposes of previous head
                    pop_fill(2 if fill_ok else 0)
                    pend_pv = make_pv(qi, h, pes)
            # drain: last head's PV + transposes, then final strip
            for pv in pend_pv[0]:
                pv()
                pop_fill(2)
            pend_pv[1]()
            cqueue.extend(make_strip(NQT - 1))
            while cqueue:
                cqueue.pop(0)()

    nc.finalize()
    return nc


def _prep_inputs(x, wq, wk, wv, wo, freqs_cos, freqs_sin):
    """Host-side shard prep. Returns in_maps for cores 0..7."""
    bf = ml_dtypes.bfloat16
    perm = np.concatenate([np.arange(0, HD, 2), np.arange(1, HD, 2)])  # rotate-half

    wq_p = (wq.astype(np.float32) / np.sqrt(HD)).reshape(DIM, N_HEADS, HD)[:, :, perm]
    wk_p = wk.astype(np.float32).reshape(DIM, N_KV, HD)[:, :, perm]

    cosT = np.ascontiguousarray(freqs_cos.astype(np.float32).T).astype(bf)
    sinT = np.ascontiguousarray(freqs_sin.astype(np.float32).T).astype(bf)

    xTs = [np.ascontiguousarray(x[b].astype(np.float32).T).astype(bf)
           for b in range(BSZ)]

    in_maps = []
    for c in range(N_CORES):
        b, g = c // 4, c % 4
        in_maps.append({
            "xT": xTs[b],
            "wq": np.ascontiguousarray(
                wq_p[:, g * H_L:(g + 1) * H_L].reshape(DIM, HL)).astype(bf),
            "wk": np.ascontiguousarray(
                wk_p[:, g * KV_L:(g + 1) * KV_L]
                .reshape(NK, 128, KV_L, HD).transpose(1, 2, 0, 3)
                .reshape(128, KV_L, NK * HD)).astype(bf),
            "wv": np.ascontiguousarray(
                wv[:, g * KV_L * HD:(g + 1) * KV_L * HD]).astype(bf),
            "wo": np.ascontiguousarray(
                wo[g * HL:(g + 1) * HL]).astype(bf),
            "cosT": cosT,
            "sinT": sinT,
        })
    return in_maps


def _run(inputs, trace=False):
    from concourse.bass_utils import run_bass_kernel_spmd

    if "nc" not in _cache:
        _cache["nc"] = _build()
    nc = _cache["nc"]

    in_maps = _prep_inputs(
        np.asarray(inputs["x"]), np.asarray(inputs["wq"]),
        np.asarray(inputs["wk"]), np.asarray(inputs["wv"]),
        np.asarray(inputs["wo"]), np.asarray(inputs["freqs_cos"]),
        np.asarray(inputs["freqs_sin"]))

    res = run_bass_kernel_spmd(nc, in_maps, core_ids=list(range(N_CORES)),
                               trace=trace)
    out = np.zeros((BSZ, SEQ, DIM), np.float32)
    for c in range(N_CORES):
        out[c // 4] += res.results[c]["out"].astype(np.float32)
    return out, res


def kernel(**inputs) -> np.ndarray:
    out, _ = _run(inputs, trace=False)
    return out


# revision 46
# speedup vs baseline: 1.2018x; 1.0201x over previous
"""Distributed GQA attention prefill kernel for 8 TRN2 NeuronCores.

Problem: llama-style attention, BSZ=2, SEQ=2048, DIM=4096, 32 Q heads,
8 KV heads, head_dim=128, causal prefill (start_pos=0, caches zero).

Sharding: data-parallel over batch (2) x tensor-parallel over heads (4).
Core c = (b, g) with b = c // 4, g = c % 4 handles batch b, Q heads
8g..8g+7, KV heads 2g..2g+1, and wo rows 1024g..1024(g+1). Each core
emits a partial [2048, 4096] output (bf16); the host sums the 4 TP
partials per batch in f32. No collectives.

On-chip layout trick: everything is computed in "transposed" layouts so
no activation transpose is ever needed:
  QT[d, t] = wq.T @ x.T       (lhsT = wq natural, rhs = xT from host)
  KT[d, t] = wk.T @ x.T
  V[t, d]  = x @ wv           (lhsT = xT chunk, rhs = wv natural)
  scoresT[kv, q] = K @ QT     (lhsT = KT tile, rhs = QT tile)
  attn[q, d+1]   = P @ [V|1]  (lhsT = expT tile, rhs = V with ones col
                               -> last column accumulates the softmax
                               denominator for free)
RoPE is applied in rotate-half form: the head_dim of wq/wk is permuted
on the host (even dims first, odd dims second) which leaves all dot
products unchanged; cos/sin arrive transposed [64, t] in bf16.

Schedule:
  Stage A streams x in 8 windows of 256 tokens.  Within a window the
  K/V projections run k-chunk-outer so the PE consumes DMA chunks as
  they land; the Q projection of window w runs during window w+1 so
  the 8MB wq load never stalls the pipe.  QT spills to DRAM.
  Stage B software-pipelines heads: the scores+exp of head h are
  interleaved with the P@V of head h-1 and with out-projection matmuls
  of the previous q-tile's tokens, so ScalarE exp latency never idles
  the PE.  Causal masking is done by zeroing the masked triangle of
  the exp output on GpSimd (no DVE op between matmul and exp).
"""

import sys

for p in ("/opt/pypackages", "/opt/trn_rl_repo"):
    if p not in sys.path:
        sys.path.insert(0, p)

import numpy as np
import ml_dtypes

BSZ, SEQ, DIM = 2, 2048, 4096
N_HEADS, N_KV, HD = 32, 8, 128
H_L, KV_L = 8, 2          # per-core local Q heads / KV heads
HL = H_L * HD             # 1024 local head dims
N_CORES = 8
WIN = 256                 # stage-A K/V sub-window (pair 0 startup)
PW = 512                  # stage-A window pair (Q/K matmul free dim)
NP = SEQ // PW
NK = DIM // 128
NQT = SEQ // 512          # attention q-tiles

_cache = {}


def _build():
    import concourse.mybir as mybir
    import concourse.tile as tile
    from concourse import bacc
    from concourse.masks import make_identity
    from contextlib import ExitStack

    f32 = mybir.dt.float32
    bf16 = mybir.dt.bfloat16
    Exp = mybir.ActivationFunctionType.Exp

    nc = bacc.Bacc()
    xT = nc.declare_dram_parameter("xT", [DIM, SEQ], bf16, isOutput=False)
    wq = nc.declare_dram_parameter("wq", [DIM, HL], bf16, isOutput=False)
    # wk arrives kh-major ([partition, kv_head, ko*hd]) so each kv head's
    # weights are one fully-contiguous 1MB DMA
    wk = nc.declare_dram_parameter("wk", [128, KV_L, NK * HD], bf16,
                                   isOutput=False)
    wv = nc.declare_dram_parameter("wv", [DIM, KV_L * HD], bf16, isOutput=False)
    wo = nc.declare_dram_parameter("wo", [HL, DIM], bf16, isOutput=False)
    cosT = nc.declare_dram_parameter("cosT", [64, SEQ], bf16, isOutput=False)
    sinT = nc.declare_dram_parameter("sinT", [64, SEQ], bf16, isOutput=False)
    out = nc.declare_dram_parameter("out", [SEQ, DIM], bf16, isOutput=True)

    qt_dram = nc.dram_tensor("qt_spill", [H_L, HD, SEQ], bf16)

    def dma_split(dst, src, n):
        """Issue n parallel DMAs over the ko axis (dim 1 of dst)."""
        ko = dst.shape[1]
        assert ko % n == 0, f"dma_split: {ko} not divisible by {n}"
        step = ko // n
        for i in range(n):
            nc.sync.dma_start(
                out=dst[:, i * step:(i + 1) * step],
                in_=src[:, i * step:(i + 1) * step])

    def dma_split_rr(dst, src, n):
        """n parallel DMAs with interleaved ko ranges: DMA i carries
        chunks i, i+n, i+2n ... so low-k chunks land first on every
        queue and the consumer (which walks k ascending) never waits
        behind a whole contiguous range."""
        for i in range(n):
            nc.sync.dma_start(out=dst[:, i::n], in_=src[:, i::n])

    with tile.TileContext(nc) as tc, ExitStack() as res:
        resid = res.enter_context(tc.tile_pool(name="resid", bufs=1))
        qt_pool = res.enter_context(tc.tile_pool(name="qt", bufs=2))
        qts = {}

        # K cache (rotated) [d, kv_tok] and V cache [kv_tok, d | ones]
        kt_all = resid.tile([128, KV_L, SEQ], bf16, tag="kt")
        v_all = resid.tile([128, SEQ // 128, KV_L, 130], bf16, tag="v")
        ident = resid.tile([128, 128], bf16, tag="ident")

        # ---- stage A: Q/K/V projection + RoPE --------------------------
        with ExitStack() as sa:
            psA = sa.enter_context(
                tc.tile_pool(name="psA", bufs=8, space="PSUM"))
            wq_sb = sa.enter_context(tc.tile_pool(name="wq", bufs=1)).tile(
                [128, NK, HL], bf16, tag="wq")
            wk_sb = sa.enter_context(tc.tile_pool(name="wk", bufs=1)).tile(
                [128, KV_L, NK * HD], bf16, tag="wk")
            wv_sb = sa.enter_context(tc.tile_pool(name="wv", bufs=1)).tile(
                [128, NK, KV_L * HD], bf16, tag="wv")
            cs_pool = sa.enter_context(tc.tile_pool(name="cs", bufs=1))
            cos_sb = cs_pool.tile([64, SEQ], bf16, tag="cos")
            sin_sb = cs_pool.tile([64, SEQ], bf16, tag="sin")
            xt_pool = sa.enter_context(tc.tile_pool(name="xt", bufs=2))
            rope_pool = sa.enter_context(tc.tile_pool(name="rope", bufs=1))
            qsp_pool = sa.enter_context(tc.tile_pool(name="qsp", bufs=2))

            def rope(ps, dst, t0, tw):
                """dst[0:64]=e*c-o*s ; dst[64:128]=e*s+o*c.
                One ScalarE copy stages the PSUM to bf16 SBUF: the PSUM
                bank frees after ~0.5us (instead of being held across
                four DVE reads) and the DVE math runs all-bf16 at 2x."""
                ste = rope_pool.tile([64, PW], bf16, tag="ste",
                                     name="ste")[:, :tw]
                sto = rope_pool.tile([64, PW], bf16, tag="sto",
                                     name="sto")[:, :tw]
                nc.scalar.copy(ste, ps[0:64, :tw])
                nc.scalar.copy(sto, ps[64:128, :tw])
                c = cos_sb[:, t0:t0 + tw]
                s = sin_sb[:, t0:t0 + tw]
                t1 = rope_pool.tile([64, PW], bf16, tag="r1", name="r1")[:, :tw]
                t2 = rope_pool.tile([64, PW], bf16, tag="r2", name="r2")[:, :tw]
                nc.vector.tensor_mul(t1, ste, c)
                nc.vector.tensor_mul(t2, sto, s)
                nc.vector.tensor_sub(dst[0:64, :tw], t1, t2)
                # r1/r2 slots reused: allocate only after the sub above is
                # emitted so the WAR dependency protects t1/t2
                t3 = rope_pool.tile([64, PW], bf16, tag="r1", name="r3")[:, :tw]
                t4 = rope_pool.tile([64, PW], bf16, tag="r2", name="r4")[:, :tw]
                nc.vector.tensor_mul(t3, ste, s)
                nc.vector.tensor_mul(t4, sto, c)
                nc.vector.tensor_add(dst[64:128, :tw], t3, t4)

            # -- DMA priority chain: pair-0's first token half + wk
            # kv-head 0 lead so the first matmuls start ASAP, then the
            # rest of pair 0, wv, cos/sin, pair 1, then the 8MB wq.
            xt_tiles = {}
            xt0 = xt_pool.tile([128, NK, PW], bf16, tag="xt", name="xt0")
            xt_tiles[0] = xt0
            xt0_src = xT[:, 0:PW].rearrange("(ko p) t -> p ko t", p=128)
            wv_src = wv.rearrange("(ko p) d -> p ko d", p=128)
            nc.sync.dma_start(out=xt0[:, 0:2, 0:WIN],
                              in_=xt0_src[:, 0:2, 0:WIN])
            nc.sync.dma_start(out=wk_sb[:, 0, 0:2 * HD], in_=wk[:, 0, 0:2 * HD])
            dma_split(xt0[:, 2:, 0:WIN], xt0_src[:, 2:, 0:WIN], 6)
            for i in range(3):
                nc.sync.dma_start(
                    out=wk_sb[:, 0, 2 * HD + i * 10 * HD:
                              min(NK * HD, 2 * HD + (i + 1) * 10 * HD)],
                    in_=wk[:, 0, 2 * HD + i * 10 * HD:
                           min(NK * HD, 2 * HD + (i + 1) * 10 * HD)])
            for i in range(3):
                nc.sync.dma_start(
                    out=wk_sb[:, 1, i * 11 * HD:min(NK * HD, (i + 1) * 11 * HD)],
                    in_=wk[:, 1, i * 11 * HD:min(NK * HD, (i + 1) * 11 * HD)])
            dma_split(xt0[:, :, WIN:PW], xt0_src[:, :, WIN:PW], 4)
            dma_split(wv_sb, wv_src, 4)
            nc.sync.dma_start(out=cos_sb, in_=cosT[:, :])
            nc.sync.dma_start(out=sin_sb, in_=sinT[:, :])
            # wq in 4 column blocks (2 heads each), BEFORE pair 1's x so
            # Q of pair 0 starts as soon as the first block lands.
            wq_src = wq.rearrange("(ko p) d -> p ko d", p=128)
            for b in range(4):
                nc.sync.dma_start(
                    out=wq_sb[:, :, b * 256:(b + 1) * 256],
                    in_=wq_src[:, :, b * 256:(b + 1) * 256])
            xt1 = xt_pool.tile([128, NK, PW], bf16, tag="xt", name="xt1")
            xt_tiles[1] = xt1
            dma_split(xt1, xT[:, PW:2 * PW].rearrange(
                "(ko p) t -> p ko t", p=128), 8)
            make_identity(nc, ident)
            nc.vector.memset(v_all[:, :, :, 128:129], 1.0)
            # warm the ScalarE Exp table now so the first attention exp
            # doesn't pay the ~1.3us ACT_TABLE_LOAD mid-kernel
            warm = rope_pool.tile([128, 1], f32, tag="warm", name="warm")
            nc.vector.memset(warm, 0.0)
            nc.scalar.activation(warm, warm, Exp)

            def k_proj_halves(xt, t0):
                """Pair 0 K: half-serial N=256 chains in separate banks,
                ordered (kh0,h0) (kh1,h0) (kh0,h1) (kh1,h1) to match the
                startup DMA arrival order (x half 0, wk kh1, x half 1)."""
                for half in range(2):
                    c0 = half * WIN
                    for kh in range(KV_L):
                        ps = psA.tile([128, WIN], f32, tag="psA",
                                      name=f"psk{kh}{half}")
                        for k in range(NK):
                            nc.tensor.matmul(
                                ps, wk_sb[:, kh, k * HD:(k + 1) * HD],
                                xt[:, k, c0:c0 + WIN],
                                start=(k == 0), stop=(k == NK - 1))
                        rope(ps, kt_all[:, kh, t0 + c0:t0 + c0 + WIN],
                             t0 + c0, WIN)

            def k_proj(xt, t0):
                # k-outer at N=512: consume each x/wk chunk as it lands
                pss = [psA.tile([128, PW], f32, tag="psA", name=f"psk{kh}")
                       for kh in range(KV_L)]
                for k in range(NK):
                    for kh in range(KV_L):
                        nc.tensor.matmul(
                            pss[kh], wk_sb[:, kh, k * HD:(k + 1) * HD],
                            xt[:, k], start=(k == 0), stop=(k == NK - 1))
                for kh in range(KV_L):
                    rope(pss[kh], kt_all[:, kh, t0:t0 + PW], t0, PW)

            def v_proj(xt, t0):
                pss = [psA.tile([128, KV_L * HD], f32, tag="psA",
                                name=f"psv{tc_}")
                       for tc_ in range(PW // 128)]
                for k in range(NK):
                    for tc_ in range(PW // 128):
                        nc.tensor.matmul(
                            pss[tc_], xt[:, k, tc_ * 128:(tc_ + 1) * 128],
                            wv_sb[:, k], start=(k == 0), stop=(k == NK - 1))
                for tc_ in range(PW // 128):
                    for kh in range(KV_L):
                        nc.scalar.copy(
                            v_all[:, t0 // 128 + tc_, kh, 0:128],
                            pss[tc_][:, kh * HD:(kh + 1) * HD])

            def q_unit(xt, t0, h):
                ps = psA.tile([128, PW], f32, tag="psA", name="psq")
                for k in range(NK):
                    nc.tensor.matmul(
                        ps, wq_sb[:, k, h * HD:(h + 1) * HD],
                        xt[:, k], start=(k == 0), stop=(k == NK - 1))
                qs = qsp_pool.tile([128, PW], bf16, tag="qs", name="qs")
                rope(ps, qs, t0, PW)
                nc.sync.dma_start(out=qt_dram[h, :, t0:t0 + PW], in_=qs)

            for p in range(NP):
                t0 = p * PW
                if p >= 1 and p + 1 < NP:
                    # prefetch next pair's x one full pair ahead
                    xt = xt_pool.tile([128, NK, PW], bf16, tag="xt",
                                      name="xt")
                    xt_tiles[p + 1] = xt
                    dma_split(xt, xT[:, t0 + PW:t0 + 2 * PW].rearrange(
                        "(ko p) t -> p ko t", p=128), 8)
                if p == 1:
                    # pair 0's Q spill is complete -> prefetch its q-tile
                    qts[0] = qt_pool.tile([128, H_L, 512], bf16,
                                          tag="qt", name="qt0")
                    dma_split(qts[0], qt_dram[:, :, 0:PW]
                              .rearrange("h p q -> p h q"), 2)
                if p == 0:
                    k_proj_halves(xt_tiles[p], t0)
                else:
                    k_proj(xt_tiles[p], t0)
                v_proj(xt_tiles[p], t0)
                for h in range(H_L):
                    q_unit(xt_tiles[p], t0, h)
                if p >= 1:
                    del xt_tiles[p - 1]

        # ---- stage B: attention with out-projection interleaved --------
        with ExitStack() as bc:
            # creation order oc -> ap -> sc: the scores pool (used first,
            # right at the stage boundary) should land on the PSUM banks
            # freed earliest by the last stage-A q_units.
            ps_oc = bc.enter_context(
                tc.tile_pool(name="ps_oc", bufs=3, space="PSUM"))
            ps_ap = bc.enter_context(
                tc.tile_pool(name="ps_ap", bufs=2, space="PSUM"))
            ps_sc = bc.enter_context(
                tc.tile_pool(name="ps_sc", bufs=3, space="PSUM"))
            exp_pool = bc.enter_context(tc.tile_pool(name="exp", bufs=32))
            asb_pool = bc.enter_context(tc.tile_pool(name="asb", bufs=8))
            rec_pool = bc.enter_context(tc.tile_pool(name="rec", bufs=8))
            at_sb = bc.enter_context(tc.tile_pool(name="at", bufs=1)).tile(
                [128, H_L, SEQ], bf16, tag="at")
            wo_pool = bc.enter_context(tc.tile_pool(name="wo", bufs=2))
            out_pool = bc.enter_context(tc.tile_pool(name="outp", bufs=4))

            wo_cur = [None]
            cqueue = []

            def make_strip(qs_):
                """Emission closures for out-proj of token strip qs_.
                wo tile loads run one di-block ahead of their consumers."""
                def load_wo(di):
                    wot = wo_pool.tile([128, H_L, 512], bf16, tag="wo",
                                       name="wot")
                    dma_split(wot, wo[:, di * 512:(di + 1) * 512].rearrange(
                        "(ho p) d -> p ho d", p=128), 2)
                    return wot

                wo_ring = [None, None]

                cls = []

                def first_load():
                    wo_ring[0] = load_wo(0)
                cls.append(first_load)
                for di in range(DIM // 512):
                    if di + 1 < DIM // 512:
                        def next_load(di=di):
                            wo_ring[(di + 1) % 2] = load_wo(di + 1)
                        cls.append(next_load)
                    for tj in range(4):
                        def pair(di=di, ti=qs_ * 4 + tj):
                            wot = wo_ring[di % 2]
                            ps = ps_oc.tile([128, 512], f32, tag="oc",
                                            name="pso")
                            for ho in range(H_L):
                                nc.tensor.matmul(
                                    ps, at_sb[:, ho, ti * 128:(ti + 1) * 128],
                                    wot[:, ho], start=(ho == 0),
                                    stop=(ho == H_L - 1))
                            osb = out_pool.tile([128, 512], bf16, tag="osb",
                                                name="osb")
                            nc.scalar.copy(osb, ps)
                            nc.sync.dma_start(
                                out=out[ti * 128:(ti + 1) * 128,
                                        di * 512:(di + 1) * 512],
                                in_=osb)
                        cls.append(pair)
                return cls

            def pop_fill(n):
                for _ in range(n):
                    if cqueue:
                        cqueue.pop(0)()

            def make_scores(qi, h):
                """Emit per-kvt closures: scores MM + exp + causal zero."""
                q0 = qi * 512
                kh = h // 4
                qt = qts[qi]
                pes = []
                cls = []
                for kvt in range(4 * (qi + 1)):
                    r = kvt - 4 * qi
                    c0 = max(r, 0) * 128
                    pe = exp_pool.tile([128, 512], bf16, tag="exp", name="pe")
                    pes.append(pe)

                    def sc(kvt=kvt, r=r, c0=c0, pe=pe):
                        ps = ps_sc.tile([128, 512], f32, tag="sc", name="pss")
                        nc.tensor.matmul(
                            ps[:, c0:],
                            kt_all[:, kh, kvt * 128:(kvt + 1) * 128],
                            qt[:, h, c0:], start=True, stop=True)
                        nc.scalar.activation(pe[:, c0:], ps[:, c0:], Exp)
                        if r >= 0:
                            # zero the causally-masked upper triangle of
                            # the diagonal 128x128 block (q < kv)
                            nc.gpsimd.affine_select(
                                out=pe[:, c0:c0 + 128],
                                in_=pe[:, c0:c0 + 128],
                                pattern=[[1, 128]],
                                compare_op=mybir.AluOpType.is_ge,
                                fill=0.0, base=0, channel_multiplier=-1)
                    cls.append(sc)
                return pes, cls

            def make_pv(qi, h, pes):
                """Per-qc closures: P@V chain + normalize; then transposes."""
                q0 = qi * 512
                kh = h // 4
                asbs = []
                cls = []
                for qc in range(4):
                    def pv(qc=qc):
                        ap = ps_ap.tile([128, 129], f32, tag="ap", name="ap")
                        last = 4 * qi + qc
                        for kvt in range(last + 1):
                            nc.tensor.matmul(
                                ap, pes[kvt][:, qc * 128:(qc + 1) * 128],
                                v_all[:, kvt, kh, 0:129],
                                start=(kvt == 0), stop=(kvt == last))
                        rec = rec_pool.tile([128, 1], f32, tag="rec",
                                            name="rec")
                        nc.vector.reciprocal(rec, ap[:, 128:129])
                        asb = asb_pool.tile([128, 128], bf16, tag="asb",
                                            name="asb")
                        nc.vector.tensor_scalar_mul(asb, ap[:, 0:128], rec)
                        asbs.append(asb)
                    cls.append(pv)

                def transp():
                    for qc in range(4):
                        pst = ps_oc.tile([128, 128], bf16, tag="oc",
                                         name="pst")
                        nc.tensor.transpose(pst, asbs[qc], ident)
                        nc.vector.tensor_copy(
                            at_sb[:, h, q0 + qc * 128:q0 + (qc + 1) * 128],
                            pst)
                return cls, transp

            pend_pv = None   # (pv closures, transp closure) of prev head

            for qi in range(NQT):
                if qi + 1 < NQT:
                    # prefetch the next q-tile during this tile's attention
                    qts[qi + 1] = qt_pool.tile([128, H_L, 512], bf16,
                                               tag="qt", name="qtn")
                    dma_split(qts[qi + 1],
                              qt_dram[:, :, (qi + 1) * PW:(qi + 2) * PW]
                              .rearrange("h p q -> p h q"), 2)
                for h in range(H_L):
                    if h == 0 and qi > 0:
                        # make the strip available now but only pop its two
                        # wo prefetch loads (pure DMA); its out-proj matmuls
                        # must wait for head 7's transposes below.
                        cqueue.extend(make_strip(qi - 1))
                        pop_fill(2)
                    fill_ok = h > 0 or qi == 0
                    pes, scs = make_scores(qi, h)
                    pvs = list(pend_pv[0]) if pend_pv else []
                    # interleave: scores tiles paced by exp; PV chains and
                    # out-proj fillers of the previous strip fill the PE
                    # while ScalarE catches up.
                    n = len(scs)
                    s = max(1, (n - 2) // 4)
                    pv_at = {2, 2 + s, 2 + 2 * s, 2 + 3 * s}
                    for i, sc in enumerate(scs):
                        sc()
                        if i in pv_at and pvs:
                            pvs.pop(0)()
                            if fill_ok:
                                pop_fill(1)
                    for pv in pvs:
                        pv()
                        if fill_ok:
                            pop_fill(1)
                    if pend_pv:
                        pend_pv[1]()   # transposes of previous head
                    pop_fill(2 if fill_ok else 0)
                    pend_pv = make_pv(qi, h, pes)
            # drain: last head's PV + transposes, then final strip
            for pv in pend_pv[0]:
                pv()
                pop_fill(2)
            pend_pv[1]()
            cqueue.extend(make_strip(NQT - 1))
            while cqueue:
                cqueue.pop(0)()

    nc.finalize()
    return nc


def _prep_inputs(x, wq, wk, wv, wo, freqs_cos, freqs_sin):
    """Host-side shard prep. Returns in_maps for cores 0..7."""
    bf = ml_dtypes.bfloat16
    perm = np.concatenate([np.arange(0, HD, 2), np.arange(1, HD, 2)])  # rotate-half

    wq_p = (wq.astype(np.float32) / np.sqrt(HD)).reshape(DIM, N_HEADS, HD)[:, :, perm]
    wk_p = wk.astype(np.float32).reshape(DIM, N_KV, HD)[:, :, perm]

    cosT = np.ascontiguousarray(freqs_cos.astype(np.float32).T).astype(bf)
    sinT = np.ascontiguousarray(freqs_sin.astype(np.float32).T).astype(bf)

    xTs = [np.ascontiguousarray(x[b].astype(np.float32).T).astype(bf)
           for b in range(BSZ)]

    in_maps = []
    for c in range(N_CORES):
        b, g = c // 4, c % 4
        in_maps.append({
            "xT": xTs[b],
            "wq": np.ascontiguousarray(
                wq_p[:, g * H_L:(g + 1) * H_L].reshape(DIM, HL)).astype(bf),
            "wk": np.ascontiguousarray(
                wk_p[:, g * KV_L:(g + 1) * KV_L]
                .reshape(NK, 128, KV_L, HD).transpose(1, 2, 0, 3)
                .reshape(128, KV_L, NK * HD)).astype(bf),
            "wv": np.ascontiguousarray(
                wv[:, g * KV_L * HD:(g + 1) * KV_L * HD]).astype(bf),
            "wo": np.ascontiguousarray(
                wo[g * HL:(g + 1) * HL]).astype(bf),
            "cosT": cosT,
            "sinT": sinT,
        })
    return in_maps


def _run(inputs, trace=False):
    from concourse.bass_utils import run_bass_kernel_spmd

    if "nc" not in _cache:
        _cache["nc"] = _build()
    nc = _cache["nc"]

    in_maps = _prep_inputs(
        np.asarray(inputs["x"]), np.asarray(inputs["wq"]),
        np.asarray(inputs["wk"]), np.asarray(inputs["wv"]),
        np.asarray(inputs["wo"]), np.asarray(inputs["freqs_cos"]),
        np.asarray(inputs["freqs_sin"]))

    res = run_bass_kernel_spmd(nc, in_maps, core_ids=list(range(N_CORES)),
                               trace=trace)
    out = np.zeros((BSZ, SEQ, DIM), np.float32)
    for c in range(N_CORES):
        out[c // 4] += res.results[c]["out"].astype(np.float32)
    return out, res


def kernel(**inputs) -> np.ndarray:
    out, _ = _run(inputs, trace=False)
    return out
